# revision 21
# baseline (speedup 1.0000x reference)
"""Trainium2 Bass kernel for nn_ContextEncoder (banded local attention encoder).

Reference computation (B=2, T=2048, D=512, H=8, dh=64, band half-width 32):
  xn   = LayerNorm(x) * g + b
  q    = ((xn @ Wp.T + bp) @ Wq.T + bq) / sqrt(dh)      per-head [B,T,H,dh]
  k, v = xn @ Wk.T + bk, xn @ Wv.T + bv
  s    = banded scores  (|i-j| <= 32), softmax over window
  ctx  = (a @ v_window) @ Wo.T + bo
  gate = sigmoid([x, ctx] @ Wg.T + bg)
  out  = x * (1 - gate) + ctx * gate

Sharding: sequence-parallel, 8 cores = 2 batches x 4 chunks of 512 tokens.
Each core gets its 512-token chunk plus a 32-token halo on each side
(zero-padded at sequence edges; per-core masks kill invalid positions),
computes its 512 output rows fully independently (no collectives), and the
host concatenates.

Algebraic folds done on host:
  - Wp folded into Wq:  q = xn @ (Wq Wp).T * s  -- removes a DxD projection.
  - k-bias dropped: a per-feature constant added to every key shifts each
    query's scores uniformly, which softmax cancels.
  - v-bias folded into bo/bg (ctx picks up exactly +bv after normalization).
  - LN gain/bias folded into weights; gate projections of x and ctx share
    one PSUM accumulation (gate_pre = x@Wg1.T + ctx@(Wg2 Wo).T + const).

Device pipeline (per core):
  - LayerNorm token-major (bn_stats) emitted FIRST so its ACT sqrt is not
    stuck behind DMA issues; x split across the sync+gpsimd queues.
  - PE-transpose xn -> xnT; q written into the zero-padded head-pair layout
    q2 so scores batch head pairs (one kT stationary, N=256 moving).
  - Scores S^T[w, q]; exp on ACT; contiguous pre-duplicated bf16 masks.
  - AV with V stationary -> ctx feature-major (no transposes), heads
    interleaved into aligned PE quadrants via a host permutation of
    Wv/Wo/Wg2.  Denominators via an all-ones stationary (broadcasts den to
    every partition); reciprocal_approx_fast + multiply normalizes into the
    O-projection operand layout.
  - Attention is software-pipelined: block b's scores are emitted before
    block b-1's AV/epilogue so the PE queue never waits on the softmax
    chain, keeping the PE p-state ramped.
  - sigmoid computed as 1/(1+exp(-z)) reusing the Exp table (the ACT engine
    holds two tables; Copy+Exp stay resident, zero reloads in steady state).
  - bf16 store (host upcasts), column-half pipelined tail.
"""

import numpy as np
import ml_dtypes

B, T, D = 2, 2048, 512
H, DH = 8, 64
WCTX = 32
NCORES = 8
CHUNK = 512          # tokens per core
NBLK = CHUNK // 128  # 4 query blocks per core
HALO = CHUNK + 2 * WCTX   # 576 tokens incl. halo
XROWS = 640          # x dram rows: 512 central + 32 left + 32 right + 64 pad
BF16 = ml_dtypes.bfloat16

_CACHE = {}


def _build_program(flags):
    """Builds the single-core Bass/Tile program (shared SPMD across 8 cores).

    flags: (bq_nz, bo_nz, bg_nz) -> emit optional bias adds.
    """
    import concourse.bass as bass
    import concourse.tile as tile
    import concourse.mybir as mybir
    from concourse import bacc

    f32 = mybir.dt.float32
    bf16 = mybir.dt.bfloat16
    AF = mybir.ActivationFunctionType
    ALU = mybir.AluOpType
    bq_nz, bo_nz, bg_nz = flags

    nc = bacc.Bacc(
        "TRN2",
        target_bir_lowering=False,
        debug=False,
        enable_asserts=False,
        num_devices=NCORES,
    )

    x_in = nc.dram_tensor("x", [XROWS, D], bf16, kind="ExternalInput")
    xt_in = nc.dram_tensor("xt", [D, CHUNK], bf16, kind="ExternalInput")
    mA_in = nc.dram_tensor("mA", [128, NBLK, 512], bf16, kind="ExternalInput")
    mB_in = nc.dram_tensor("mB", [64, NBLK, 512], bf16, kind="ExternalInput")
    w_in = {
        n: nc.dram_tensor(n, [D, D], bf16, kind="ExternalInput")
        for n in ["wq", "wk", "wv", "wo", "wg1", "wg2"]
    }
    bqh_in = nc.dram_tensor("bqh", [128, 4], f32, kind="ExternalInput")
    bo_in = nc.dram_tensor("bo", [1, D], f32, kind="ExternalInput")
    bg_in = nc.dram_tensor("bg", [1, D], f32, kind="ExternalInput")
    out_t = nc.dram_tensor("out", [CHUNK, D], bf16, kind="ExternalOutput")

    with tile.TileContext(nc) as tc:
        with (
            tc.tile_pool(name="wpool", bufs=1) as wpool,
            tc.tile_pool(name="apool", bufs=1) as apool,
            tc.tile_pool(name="small", bufs=1) as small,
            tc.tile_pool(name="stats", bufs=6) as stats_pool,
            tc.tile_pool(name="attn", bufs=4) as attn_pool,
            tc.tile_pool(name="rp", bufs=2) as rp_pool,
            tc.tile_pool(name="fin", bufs=2) as fin_pool,
            tc.tile_pool(name="pj", bufs=2, space="PSUM") as pj,
            tc.tile_pool(name="sp0", bufs=2, space="PSUM") as sp0,
            tc.tile_pool(name="sp1", bufs=1, space="PSUM") as sp1,
            tc.tile_pool(name="cp", bufs=2, space="PSUM") as cp,
            tc.tile_pool(name="bcp", bufs=1, space="PSUM") as bcp,
        ):
            # ---- persistent SBUF tensors ----
            x_sb = apool.tile([128, 5, D], bf16, tag="x")
            xn0 = apool.tile([128, 5, D], bf16, tag="xn0")
            xnT = apool.tile([128, 4, HALO], bf16, tag="xnT")
            q2 = apool.tile([128, 4, 2, CHUNK], bf16, tag="q2")
            kT = apool.tile([128, 4, HALO], bf16, tag="kT")
            v_sb = apool.tile([128, 5, D], bf16, tag="v")
            xt_sb = apool.tile([128, 4, CHUNK], bf16, tag="xt")
            mA_sb = apool.tile([128, NBLK, 512], bf16, tag="mA")
            mB_sb = apool.tile([64, NBLK, 512], bf16, tag="mB")
            ctxT = apool.tile([128, 4, NBLK, 128], bf16, tag="ctxT")

            ws = {n: wpool.tile([128, 4, D], bf16, tag=n, name=n) for n in w_in}
            eps_t = small.tile([128, 1], f32, tag="eps")
            ones_sb = small.tile([128, 128], bf16, tag="ones")
            ident = small.tile([128, 128], bf16, tag="ident")

            # ---- input DMAs ----
            def wdma(eng, name):
                eng.dma_start(
                    out=ws[name][:],
                    in_=w_in[name][:].rearrange("(c p) d -> p c d", p=128),
                )

            # x split by whole token-tiles across the two HW queues so each
            # queue completes tiles independently (no cross-queue join per
            # tile); wq rides the gpsimd SW queue (it has slack).
            x_view = x_in[:].rearrange("(c p) d -> p c d", p=128)
            nc.sync.dma_start(out=x_sb[:, 0, :], in_=x_view[:, 0, :])
            nc.scalar.dma_start(out=x_sb[:, 1, :], in_=x_view[:, 1, :])
            nc.sync.dma_start(out=x_sb[:, 2, :], in_=x_view[:, 2, :])
            nc.scalar.dma_start(out=x_sb[:, 3, :], in_=x_view[:, 3, :])
            nc.scalar.dma_start(out=x_sb[:, 4, :], in_=x_view[:, 4, :])
            wdma(nc.gpsimd, "wq")
            wdma(nc.sync, "wk")
            wdma(nc.sync, "wv")
            nc.sync.dma_start(out=mA_sb[:], in_=mA_in[:])
            from concourse.masks import make_identity
            make_identity(nc, ident[:])
            nc.gpsimd.memset(ones_sb[:], 1.0)
            nc.vector.memset(eps_t[:], 1e-5)
            # zero the dead half of each q2 (pair, evenodd) slot once
            for p in range(4):
                nc.gpsimd.memset(q2[64:128, p, 0, :], 0.0)
                nc.gpsimd.memset(q2[0:64, p, 1, :], 0.0)

            bqh = bo_bc = bg_bc = None
            if bq_nz:
                bqh = small.tile([128, 4], f32, tag="bqh")
                nc.gpsimd.dma_start(out=bqh[:], in_=bqh_in[:])
            if bo_nz:
                bo_bc = small.tile([128, D], f32, tag="bo_bc")
                nc.gpsimd.dma_start(out=bo_bc[:], in_=bo_in[:].to_broadcast([128, D]))
            if bg_nz:
                bg_bc = small.tile([128, D], f32, tag="bg_bc")
                nc.gpsimd.dma_start(out=bg_bc[:], in_=bg_in[:].to_broadcast([128, D]))

            # ---- stage 1: LayerNorm (token-major); emitted before any
            # other scalar-engine work so the ACT sqrt runs as data lands
            for t in [0, 1, 2, 3, 4]:
                rows = 128 if t < 4 else 64
                st = stats_pool.tile([128, 6], f32, tag="st")
                mv = stats_pool.tile([128, 2], f32, tag="mv")
                rstd = stats_pool.tile([128, 1], f32, tag="rstd")
                nc.vector.bn_stats(out=st[:rows], in_=x_sb[:rows, t, :])
                nc.vector.bn_aggr(out=mv[:rows], in_=st[:rows])
                nc.scalar.activation(
                    out=rstd[:rows], in_=mv[:rows, 1:2], func=AF.Sqrt,
                    bias=eps_t[:rows], scale=1.0,
                )
                nc.vector.reciprocal(out=rstd[:rows], in_=rstd[:rows])
                nc.vector.tensor_scalar(
                    out=xn0[:rows, t, :], in0=x_sb[:rows, t, :],
                    scalar1=mv[:rows, 0:1], scalar2=rstd[:rows],
                    op0=ALU.subtract, op1=ALU.mult,
                )

            # late inputs issued on the scalar queue AFTER the LN chain so
            # they cannot delay the sqrt; they are only needed mid-kernel
            nc.scalar.dma_start(out=mB_sb[:], in_=mB_in[:])
            nc.scalar.dma_start(
                out=xt_sb[:], in_=xt_in[:].rearrange("(c p) d -> p c d", p=128)
            )
            wdma(nc.scalar, "wg1")
            wdma(nc.scalar, "wo")
            wdma(nc.scalar, "wg2")

            # ---- stage 2: transpose xn0 -> xnT via PE (halo-frame order) ----
            # x rows: [0:512] central (halo 32..544), [512:544] left halo
            # (halo 0..32), [544:576] right halo (halo 544..576)
            # central transposes first (need only x tiles 0-3); the t=4
            # halo transposes go after so they never stall the PE queue
            for j in range(4):
                tp = pj.tile([128, 512], bf16, tag="pj", name=f"tpx{j}")
                for t in range(4):
                    nc.tensor.transpose(
                        tp[:, 128 * t: 128 * (t + 1)],
                        xn0[:, t, 128 * j: 128 * (j + 1)],
                        ident[:],
                    )
                nc.vector.tensor_copy(out=xnT[:, j, 32:544], in_=tp[:])
            for j in range(4):
                th = pj.tile([128, 512], bf16, tag="pj", name=f"thx{j}")
                nc.tensor.transpose(
                    th[:, 0:64],
                    xn0[0:64, 4, 128 * j: 128 * (j + 1)],
                    ident[0:64, 0:64],
                )
                _base = xnT[:, j, :]
                halo_out = bass.AP(
                    tensor=_base.tensor,
                    offset=_base.offset,
                    ap=[list(_base.ap[0]), [544, 2], [1, 32]],
                )
                nc.vector.tensor_copy(
                    out=halo_out, in_=th[:, 0:64].rearrange("p (a b) -> p a b", b=32)
                )

            # ---- stage 3: projections ----
            # q: folded single projection, written into the zero-padded pair
            # layout (head-even rows -> partitions 0-63, head-odd -> 64-127)
            for j in range(4):
                ps = pj.tile([128, 512], f32, tag="pj")
                for c in range(4):
                    nc.tensor.matmul(
                        ps[:], ws["wq"][:, c, 128 * j: 128 * (j + 1)],
                        xnT[:, c, 32: 32 + CHUNK],
                        start=(c == 0), stop=(c == 3),
                    )
                if bq_nz:
                    nc.vector.tensor_scalar(
                        out=q2[0:64, j, 0, :], in0=ps[0:64],
                        scalar1=bqh[0:64, j: j + 1], scalar2=None, op0=ALU.add,
                    )
                    nc.vector.tensor_scalar(
                        out=q2[64:128, j, 1, :], in0=ps[64:128],
                        scalar1=bqh[64:128, j: j + 1], scalar2=None, op0=ALU.add,
                    )
                else:
                    nc.vector.tensor_copy(out=q2[0:64, j, 0, :], in_=ps[0:64])
                    nc.vector.tensor_copy(out=q2[64:128, j, 1, :], in_=ps[64:128])
            # kT[d, w] = Wk_eff @ xnT  (all 576 halo tokens; k-bias dropped)
            for j in range(4):
                ps = pj.tile([128, 512], f32, tag="pj")
                ps2 = pj.tile([128, 512], f32, tag="pj")
                for c in range(4):
                    nc.tensor.matmul(
                        ps[:], ws["wk"][:, c, 128 * j: 128 * (j + 1)],
                        xnT[:, c, 0:512],
                        start=(c == 0), stop=(c == 3),
                    )
                for c in range(4):
                    nc.tensor.matmul(
                        ps2[:, 0:64], ws["wk"][:, c, 128 * j: 128 * (j + 1)],
                        xnT[:, c, 512:576],
                        start=(c == 0), stop=(c == 3),
                    )
                nc.scalar.activation(out=kT[:, j, 0:512], in_=ps[:], func=AF.Copy)
                nc.scalar.activation(
                    out=kT[:, j, 512:576], in_=ps2[:, 0:64], func=AF.Copy
                )
            # v token-major (feature-permuted Wv; bias folded into bo/bg)
            for t in range(5):
                rows = 128 if t < 4 else 64
                ps = pj.tile([128, 512], f32, tag="pj")
                for c in range(4):
                    nc.tensor.matmul(
                        ps[:rows], xnT[:, c, 128 * t: 128 * t + rows],
                        ws["wv"][:, c, :],
                        start=(c == 0), stop=(c == 3),
                    )
                nc.scalar.activation(
                    out=v_sb[:rows, t, :], in_=ps[:rows], func=AF.Copy
                )

            # ---- stage 4: software-pipelined banded attention ----
            # stage st emits scores+exp for block st, then the AV/normalize
            # consumers for block st-1, then block st's masks, then block
            # st-1's epilogue.  The in-order PE queue always has independent
            # work while the softmax chain of the other block drains, and the
            # DVE sees the normalize ops ahead of the next masks.
            a_tiles = {}
            for st in range(NBLK + 1):
                if st < NBLK:
                    b = st
                    for g in range(2):
                        s0 = sp0.tile([128, 2, 256], f32, tag="s0")
                        s1 = sp1.tile([64, 2, 256], f32, tag="s1")
                        for pr in range(2):
                            p = 2 * g + pr
                            q_ap = q2[:, p, :, 128 * b: 128 * (b + 1)]
                            nc.tensor.matmul(
                                s0[:, pr, :],
                                kT[:, p, 128 * b: 128 * b + 128],
                                q_ap, start=True, stop=True,
                            )
                            nc.tensor.matmul(
                                s1[:, pr, :],
                                kT[:, p, 128 * b + 128: 128 * b + 192],
                                q_ap, start=True, stop=True,
                            )
                        a0 = attn_pool.tile([128, 2, 256], bf16, tag="a0")
                        a1 = attn_pool.tile([64, 2, 256], bf16, tag="a1")
                        # s1 first: its single PSUM buffer is the next
                        # score-matmul's dependency
                        nc.scalar.activation(
                            out=a1[:].rearrange("p a b -> p (a b)"),
                            in_=s1[:].rearrange("p a b -> p (a b)"), func=AF.Exp,
                        )
                        nc.scalar.activation(
                            out=a0[:].rearrange("p a b -> p (a b)"),
                            in_=s0[:].rearrange("p a b -> p (a b)"), func=AF.Exp,
                        )
                        a_tiles[(b, g)] = (a0, a1)
                if st >= 1:
                    bp_ = st - 1
                    cps = cp.tile([128, 4, 128], f32, tag="cps")
                    for g in range(2):
                        a0, a1 = a_tiles[(bp_, g)]
                        a0f = a0[:].rearrange("p a b -> p (a b)")
                        a1f = a1[:].rearrange("p a b -> p (a b)")
                        # denominators broadcast to every partition by an
                        # all-ones stationary
                        bc = bcp.tile([128, 512], f32, tag="bc")
                        nc.tensor.matmul(
                            bc[:], ones_sb[:], a0f, start=True, stop=False,
                        )
                        nc.tensor.matmul(
                            bc[:], ones_sb[0:64, :], a1f, start=False, stop=True,
                        )
                        # AV: V stationary -> ctx feature-major (head h in
                        # feature tile h%4, partition half h//4 = g)
                        po = 64 * g
                        for hh in range(4):
                            nc.tensor.matmul(
                                cps[po: po + 64, hh, :],
                                v_sb[:, bp_, 128 * hh + po: 128 * hh + po + 64],
                                a0[:, hh >> 1, 128 * (hh & 1): 128 * (hh & 1) + 128],
                                start=True, stop=False,
                            )
                            nc.tensor.matmul(
                                cps[po: po + 64, hh, :],
                                v_sb[0:64, bp_ + 1, 128 * hh + po: 128 * hh + po + 64],
                                a1[:, hh >> 1, 128 * (hh & 1): 128 * (hh & 1) + 128],
                                start=False, stop=True,
                            )
                        rbc = rp_pool.tile([64, 512], f32, tag="rbc")
                        nc.vector.reciprocal_approx_fast(
                            out=rbc[:], in_=bc[po: po + 64, :]
                        )
                        nc.vector.tensor_mul(
                            out=ctxT[po: po + 64, :, bp_, :],
                            in0=cps[po: po + 64, :, :],
                            in1=rbc[:].rearrange("p (a q) -> p a q", q=128),
                        )
                        del a_tiles[(bp_, g)]
                if st < NBLK:
                    b = st
                    for g in range(2):
                        a0, a1 = a_tiles[(b, g)]
                        nc.gpsimd.tensor_mul(
                            out=a1[:].rearrange("p a b -> p (a b)"),
                            in0=a1[:].rearrange("p a b -> p (a b)"),
                            in1=mB_sb[:, b, :],
                        )
                        nc.vector.tensor_mul(
                            out=a0[:].rearrange("p a b -> p (a b)"),
                            in0=a0[:].rearrange("p a b -> p (a b)"),
                            in1=mA_sb[:, b, :],
                        )
                if st >= 1:
                    b = st - 1
                    # ---- epilogue for block b: gate (x and ctx parts share
                    # one PSUM accumulation), O-proj, sigmoid, blend, store.
                    # Only Exp/Sigmoid touch ACT in this phase, so both
                    # tables stay resident (no reloads).
                    gacc = pj.tile([128, 512], f32, tag="pj", name=f"gacc{b}")
                    for c in range(4):
                        nc.tensor.matmul(
                            gacc[:], xt_sb[:, c, 128 * b: 128 * (b + 1)],
                            ws["wg1"][:, c, :],
                            start=(c == 0), stop=False,
                        )
                    for c in range(4):
                        nc.tensor.matmul(
                            gacc[:], ctxT[:, c, b, :], ws["wg2"][:, c, :],
                            start=False, stop=(c == 3),
                        )
                    ops = pj.tile([128, 512], f32, tag="pj")
                    for c in range(4):
                        nc.tensor.matmul(
                            ops[:], ctxT[:, c, b, :], ws["wo"][:, c, :],
                            start=(c == 0), stop=(c == 3),
                        )
                    diff = fin_pool.tile([128, 512], f32, tag="diff")
                    gate = fin_pool.tile([128, 512], f32, tag="gate")
                    outs = fin_pool.tile([128, 512], bf16, tag="outs")
                    if bo_nz:
                        nc.vector.tensor_add(
                            out=ops[:], in0=ops[:], in1=bo_bc[:]
                        )
                    nc.vector.tensor_sub(out=diff[:], in0=ops[:], in1=x_sb[:, b, :])
                    if bg_nz:
                        nc.vector.tensor_add(
                            out=gacc[:], in0=gacc[:], in1=bg_bc[:]
                        )
                    nc.scalar.activation(
                        out=gate[:], in_=gacc[:], func=AF.Sigmoid
                    )
                    # out = x + gate * (o - x), column halves pipeline the
                    # multiply/add/store tail
                    for hf in range(2):
                        hs = slice(256 * hf, 256 * (hf + 1))
                        nc.vector.tensor_mul(
                            out=diff[:, hs], in0=diff[:, hs], in1=gate[:, hs]
                        )
                        nc.vector.tensor_add(
                            out=outs[:, hs], in0=diff[:, hs], in1=x_sb[:, b, hs]
                        )
                        nc.sync.dma_start(
                            out=out_t[:].rearrange("(c p) d -> p c d", p=128)[:, b, hs],
                            in_=outs[:, hs],
                        )
    nc.compile()
    return nc


def _host_prep(inputs):
    """Fold LN gain/bias + scale + Wp + bv into weights, build per-core maps."""
    x = np.asarray(inputs["token_embeds"], np.float32)
    g = np.asarray(inputs["ln_g"], np.float32)
    lb = np.asarray(inputs["ln_b"], np.float32)
    Wp = np.asarray(inputs["Wp"], np.float32)
    Wq = np.asarray(inputs["Wq"], np.float32)
    Wk = np.asarray(inputs["Wk"], np.float32)
    Wv = np.asarray(inputs["Wv"], np.float32)
    Wo = np.asarray(inputs["Wo"], np.float32)
    Wg = np.asarray(inputs["Wg"], np.float32)
    bp = np.asarray(inputs["bp"], np.float32)
    bq = np.asarray(inputs["bq"], np.float32)
    bv = np.asarray(inputs["bv"], np.float32)
    bo = np.asarray(inputs["bo"], np.float32)
    bg = np.asarray(inputs["bg"], np.float32)

    scale = 1.0 / np.sqrt(np.float32(DH))
    # feature permutation for ctx: head h features -> tile h%4, half h//4
    perm = np.zeros(D, np.int64)
    for h in range(H):
        c, gg = h % 4, h // 4
        perm[128 * c + 64 * gg: 128 * c + 64 * gg + 64] = np.arange(
            64 * h, 64 * h + 64
        )

    Wpq = (Wq @ Wp) * scale                       # folded q projection
    wq = np.ascontiguousarray((Wpq * g[None, :]).T).astype(BF16)
    wk = np.ascontiguousarray((Wk * g[None, :]).T).astype(BF16)
    wv_p = (Wv * g[None, :])[perm, :]             # permuted output features
    wv = np.ascontiguousarray(wv_p.T).astype(BF16)
    wo = np.ascontiguousarray(Wo[:, perm].T).astype(BF16)
    wg1 = np.ascontiguousarray(Wg[:, :D].T).astype(BF16)
    # reference gates on ctx AFTER the O-projection; fold Wo into Wg2 so the
    # gate matmul can consume pre-projection (permuted) ctx directly
    Wg2o = Wg[:, D:] @ Wo
    wg2 = np.ascontiguousarray(Wg2o[:, perm].T).astype(BF16)

    bq_eff = (Wq @ (Wp @ lb + bp) + bq) * scale
    bv_eff = Wv @ lb + bv
    # device ctx omits the v-bias; it re-enters as a constant through both
    # the O-projection and the folded gate projection
    bo_eff = Wo @ bv_eff + bo
    bg_eff = Wg[:, D:] @ bo_eff + bg

    bqh = np.ascontiguousarray(bq_eff.reshape(4, 128).T).astype(np.float32)
    flags = (
        bool(np.any(bq_eff != 0)),
        bool(np.any(bo_eff != 0)),
        bool(np.any(bg_eff != 0)),
    )

    in_maps = []
    for core in range(NCORES):
        bi, ci = core // 4, core % 4
        s = ci * CHUNK
        xr = np.zeros((XROWS, D), BF16)
        xr[0:CHUNK] = x[bi, s: s + CHUNK]
        if s - WCTX >= 0:
            xr[CHUNK: CHUNK + WCTX] = x[bi, s - WCTX: s]
        if s + CHUNK + WCTX <= T:
            xr[CHUNK + WCTX: CHUNK + 2 * WCTX] = x[bi, s + CHUNK: s + CHUNK + WCTX]
        xt = np.ascontiguousarray(x[bi, s: s + CHUNK].T).astype(BF16)

        # mask[b, rr, cc]: query r=128b+rr (local), key halo pos j=128b+cc;
        # duplicated 4x along columns (pair x evenodd) so the on-device
        # multiply is a contiguous 2D bf16 op
        rr = np.arange(128)[:, None]
        cc = np.arange(192)[None, :]
        m = np.zeros((NBLK, 128, 192), np.float32)
        for qb in range(NBLK):
            band = (cc - rr >= 0) & (cc - rr <= 2 * WCTX)
            gkey = s + 128 * qb + cc - WCTX + 0 * rr
            m[qb] = (band & (gkey >= 0) & (gkey < T)).astype(np.float32)
        mA = np.ascontiguousarray(
            np.tile(m[:, :, :128].transpose(2, 0, 1), (1, 1, 4))
        ).astype(BF16)
        mB = np.ascontiguousarray(
            np.tile(m[:, :, 128:].transpose(2, 0, 1), (1, 1, 4))
        ).astype(BF16)

        in_maps.append({
            "x": xr, "xt": xt, "mA": mA, "mB": mB,
            "wq": wq, "wk": wk, "wv": wv, "wo": wo,
            "wg1": wg1, "wg2": wg2,
            "bqh": bqh,
            "bo": bo_eff.reshape(1, D).astype(np.float32),
            "bg": bg_eff.reshape(1, D).astype(np.float32),
        })
    return in_maps, flags


def _run(inputs, trace=False):
    from concourse.bass_utils import run_bass_kernel_spmd

    in_maps, flags = _host_prep(inputs)
    if flags not in _CACHE:
        _CACHE[flags] = _build_program(flags)
    nc = _CACHE[flags]
    res = run_bass_kernel_spmd(nc, in_maps, list(range(NCORES)), trace=trace)
    out = np.zeros((B, T, D), np.float32)
    for core in range(NCORES):
        bi, ci = core // 4, core % 4
        out[bi, ci * CHUNK: (ci + 1) * CHUNK] = np.asarray(
            res.results[core]["out"], dtype=np.float32
        )
    return out, res


def kernel(**inputs):
    out, _ = _run(inputs, trace=False)
    return out


# revision 22
# speedup vs baseline: 1.1907x; 1.1907x over previous
"""Trainium2 Bass kernel for nn_ContextEncoder (banded local attention encoder).

Reference computation (B=2, T=2048, D=512, H=8, dh=64, band half-width 32):
  xn   = LayerNorm(x) * g + b
  q    = ((xn @ Wp.T + bp) @ Wq.T + bq) / sqrt(dh)      per-head [B,T,H,dh]
  k, v = xn @ Wk.T + bk, xn @ Wv.T + bv
  s    = banded scores  (|i-j| <= 32), softmax over window
  ctx  = (a @ v_window) @ Wo.T + bo
  gate = sigmoid([x, ctx] @ Wg.T + bg)
  out  = x * (1 - gate) + ctx * gate

Sharding: sequence-parallel, 8 cores = 2 batches x 4 chunks of 512 tokens.
Each core gets its 512-token chunk plus a 32-token halo on each side
(zero-padded at sequence edges; per-core masks kill invalid positions),
computes its 512 output rows fully independently (no collectives), and the
host concatenates.

Algebraic folds done on host:
  - Wp folded into Wq:  q = xn @ (Wq Wp).T * s  -- removes a DxD projection.
  - k-bias dropped: a per-feature constant added to every key shifts each
    query's scores uniformly, which softmax cancels.
  - v-bias folded into bo/bg (ctx picks up exactly +bv after normalization).
  - LN gain/bias folded into weights; gate projections of x and ctx share
    one PSUM accumulation (gate_pre = x@Wg1.T + ctx@(Wg2 Wo).T + const).

Device pipeline (per core):
  - LayerNorm token-major (bn_stats) emitted FIRST so its ACT sqrt is not
    stuck behind DMA issues; x split across the sync+gpsimd queues.
  - PE-transpose xn -> xnT; q written into the zero-padded head-pair layout
    q2 so scores batch head pairs (one kT stationary, N=256 moving).
  - Scores S^T[w, q]; exp on ACT; contiguous pre-duplicated bf16 masks.
  - AV with V stationary -> ctx feature-major (no transposes), heads
    interleaved into aligned PE quadrants via a host permutation of
    Wv/Wo/Wg2.  Denominators via an all-ones stationary (broadcasts den to
    every partition); reciprocal_approx_fast + multiply normalizes into the
    O-projection operand layout.
  - Attention is software-pipelined: block b's scores are emitted before
    block b-1's AV/epilogue so the PE queue never waits on the softmax
    chain, keeping the PE p-state ramped.
  - sigmoid computed as 1/(1+exp(-z)) reusing the Exp table (the ACT engine
    holds two tables; Copy+Exp stay resident, zero reloads in steady state).
  - bf16 store (host upcasts), column-half pipelined tail.
"""

import numpy as np
import ml_dtypes

B, T, D = 2, 2048, 512
H, DH = 8, 64
WCTX = 32
NCORES = 8
CHUNK = 512          # tokens per core
NBLK = CHUNK // 128  # 4 query blocks per core
HALO = CHUNK + 2 * WCTX   # 576 tokens incl. halo
XROWS = 640          # x dram rows: 512 central + 32 left + 32 right + 64 pad
BF16 = ml_dtypes.bfloat16

_CACHE = {}


def _build_program(flags):
    """Builds the single-core Bass/Tile program (shared SPMD across 8 cores).

    flags: (bq_nz, bo_nz, bg_nz) -> emit optional bias adds.
    """
    import concourse.bass as bass
    import concourse.tile as tile
    import concourse.mybir as mybir
    from concourse import bacc

    f32 = mybir.dt.float32
    bf16 = mybir.dt.bfloat16
    AF = mybir.ActivationFunctionType
    ALU = mybir.AluOpType
    bq_nz, bo_nz, bg_nz = flags

    nc = bacc.Bacc(
        "TRN2",
        target_bir_lowering=False,
        debug=False,
        enable_asserts=False,
        num_devices=NCORES,
    )

    x_in = nc.dram_tensor("x", [XROWS, D], bf16, kind="ExternalInput")
    xt_in = nc.dram_tensor("xt", [D, CHUNK], bf16, kind="ExternalInput")
    mA_in = nc.dram_tensor("mA", [128, NBLK, 512], bf16, kind="ExternalInput")
    mB_in = nc.dram_tensor("mB", [64, NBLK, 512], bf16, kind="ExternalInput")
    w_in = {
        n: nc.dram_tensor(n, [D, D], bf16, kind="ExternalInput")
        for n in ["wq", "wk", "wv", "wo", "wg1", "wg2"]
    }
    bqh_in = nc.dram_tensor("bqh", [128, 4], f32, kind="ExternalInput")
    bo_in = nc.dram_tensor("bo", [1, D], f32, kind="ExternalInput")
    bg_in = nc.dram_tensor("bg", [1, D], f32, kind="ExternalInput")
    out_t = nc.dram_tensor("out", [CHUNK, D], bf16, kind="ExternalOutput")

    with tile.TileContext(nc) as tc:
        with (
            tc.tile_pool(name="wpool", bufs=1) as wpool,
            tc.tile_pool(name="apool", bufs=1) as apool,
            tc.tile_pool(name="small", bufs=1) as small,
            tc.tile_pool(name="stats", bufs=6) as stats_pool,
            tc.tile_pool(name="attn", bufs=4) as attn_pool,
            tc.tile_pool(name="rp", bufs=2) as rp_pool,
            tc.tile_pool(name="fin", bufs=2) as fin_pool,
            tc.tile_pool(name="pj", bufs=2, space="PSUM") as pj,
            tc.tile_pool(name="sp0", bufs=2, space="PSUM") as sp0,
            tc.tile_pool(name="sp1", bufs=1, space="PSUM") as sp1,
            tc.tile_pool(name="cp", bufs=2, space="PSUM") as cp,
            tc.tile_pool(name="bcp", bufs=1, space="PSUM") as bcp,
        ):
            # ---- persistent SBUF tensors ----
            x_sb = apool.tile([128, 5, D], bf16, tag="x")
            xn0 = apool.tile([128, 5, D], bf16, tag="xn0")
            xnT = apool.tile([128, 4, HALO], bf16, tag="xnT")
            q2 = apool.tile([128, 4, 2, CHUNK], bf16, tag="q2")
            kT = apool.tile([128, 4, HALO], bf16, tag="kT")
            v_sb = apool.tile([128, 5, D], bf16, tag="v")
            xt_sb = apool.tile([128, 4, CHUNK], bf16, tag="xt")
            mA_sb = apool.tile([128, NBLK, 512], bf16, tag="mA")
            mB_sb = apool.tile([64, NBLK, 512], bf16, tag="mB")
            ctxT = apool.tile([128, 4, NBLK, 128], bf16, tag="ctxT")

            ws = {n: wpool.tile([128, 4, D], bf16, tag=n, name=n) for n in w_in}
            eps_t = small.tile([128, 1], f32, tag="eps")
            ones_sb = small.tile([128, 128], bf16, tag="ones")
            ident = small.tile([128, 128], bf16, tag="ident")

            # ---- input DMAs ----
            def wdma(eng, name):
                eng.dma_start(
                    out=ws[name][:],
                    in_=w_in[name][:].rearrange("(c p) d -> p c d", p=128),
                )

            # x monopolizes both HW queues' in-flight slots (4 commands
            # each) so no weight command steals bandwidth from the
            # LN-critical load; wq rides the gpsimd SW queue (it has slack).
            x_view = x_in[:].rearrange("(c p) d -> p c d", p=128)
            for sl in [(0, 2), (2, 3), (3, 4), (4, 5)]:
                nc.sync.dma_start(
                    out=x_sb[:, sl[0]: sl[1], 0:256],
                    in_=x_view[:, sl[0]: sl[1], 0:256],
                )
                nc.scalar.dma_start(
                    out=x_sb[:, sl[0]: sl[1], 256:512],
                    in_=x_view[:, sl[0]: sl[1], 256:512],
                )
            wdma(nc.gpsimd, "wq")
            wdma(nc.sync, "wk")
            wdma(nc.sync, "wv")
            nc.sync.dma_start(out=mA_sb[:], in_=mA_in[:])
            from concourse.masks import make_identity
            make_identity(nc, ident[:])
            nc.gpsimd.memset(ones_sb[:], 1.0)
            nc.vector.memset(eps_t[:], 1e-5)
            # zero the dead half of each q2 (pair, evenodd) slot once
            for p in range(4):
                nc.gpsimd.memset(q2[64:128, p, 0, :], 0.0)
                nc.gpsimd.memset(q2[0:64, p, 1, :], 0.0)

            bqh = bo_bc = bg_bc = None
            if bq_nz:
                bqh = small.tile([128, 4], f32, tag="bqh")
                nc.gpsimd.dma_start(out=bqh[:], in_=bqh_in[:])
            if bo_nz:
                bo_bc = small.tile([128, D], f32, tag="bo_bc")
                nc.gpsimd.dma_start(out=bo_bc[:], in_=bo_in[:].to_broadcast([128, D]))
            if bg_nz:
                bg_bc = small.tile([128, D], f32, tag="bg_bc")
                nc.gpsimd.dma_start(out=bg_bc[:], in_=bg_in[:].to_broadcast([128, D]))

            # ---- stage 1: LayerNorm (token-major); emitted before any
            # other scalar-engine work so the ACT sqrt runs as data lands
            for t in [0, 1, 2, 3, 4]:
                rows = 128 if t < 4 else 64
                st = stats_pool.tile([128, 6], f32, tag="st")
                mv = stats_pool.tile([128, 2], f32, tag="mv")
                rstd = stats_pool.tile([128, 1], f32, tag="rstd")
                nc.vector.bn_stats(out=st[:rows], in_=x_sb[:rows, t, :])
                nc.vector.bn_aggr(out=mv[:rows], in_=st[:rows])
                nc.scalar.activation(
                    out=rstd[:rows], in_=mv[:rows, 1:2], func=AF.Sqrt,
                    bias=eps_t[:rows], scale=1.0,
                )
                nc.vector.reciprocal(out=rstd[:rows], in_=rstd[:rows])
                nc.vector.tensor_scalar(
                    out=xn0[:rows, t, :], in0=x_sb[:rows, t, :],
                    scalar1=mv[:rows, 0:1], scalar2=rstd[:rows],
                    op0=ALU.subtract, op1=ALU.mult,
                )

            # late inputs issued on the scalar queue AFTER the LN chain so
            # they cannot delay the sqrt; they are only needed mid-kernel
            nc.scalar.dma_start(out=mB_sb[:], in_=mB_in[:])
            nc.scalar.dma_start(
                out=xt_sb[:], in_=xt_in[:].rearrange("(c p) d -> p c d", p=128)
            )
            wdma(nc.scalar, "wg1")
            wdma(nc.scalar, "wo")
            wdma(nc.scalar, "wg2")

            # ---- stage 2: transpose xn0 -> xnT via PE (halo-frame order) ----
            # x rows: [0:512] central (halo 32..544), [512:544] left halo
            # (halo 0..32), [544:576] right halo (halo 544..576)
            # central transposes first (need only x tiles 0-3); the t=4
            # halo transposes go after so they never stall the PE queue
            for j in range(4):
                tp = pj.tile([128, 512], bf16, tag="pj", name=f"tpx{j}")
                for t in range(4):
                    nc.tensor.transpose(
                        tp[:, 128 * t: 128 * (t + 1)],
                        xn0[:, t, 128 * j: 128 * (j + 1)],
                        ident[:],
                    )
                nc.vector.tensor_copy(out=xnT[:, j, 32:544], in_=tp[:])
            for j in range(4):
                th = pj.tile([128, 512], bf16, tag="pj", name=f"thx{j}")
                nc.tensor.transpose(
                    th[:, 0:64],
                    xn0[0:64, 4, 128 * j: 128 * (j + 1)],
                    ident[0:64, 0:64],
                )
                _base = xnT[:, j, :]
                halo_out = bass.AP(
                    tensor=_base.tensor,
                    offset=_base.offset,
                    ap=[list(_base.ap[0]), [544, 2], [1, 32]],
                )
                nc.vector.tensor_copy(
                    out=halo_out, in_=th[:, 0:64].rearrange("p (a b) -> p a b", b=32)
                )

            # ---- stage 3: projections ----
            # q: folded single projection, written into the zero-padded pair
            # layout (head-even rows -> partitions 0-63, head-odd -> 64-127)
            for j in range(4):
                ps = pj.tile([128, 512], f32, tag="pj")
                for c in range(4):
                    nc.tensor.matmul(
                        ps[:], ws["wq"][:, c, 128 * j: 128 * (j + 1)],
                        xnT[:, c, 32: 32 + CHUNK],
                        start=(c == 0), stop=(c == 3),
                    )
                if bq_nz:
                    nc.vector.tensor_scalar(
                        out=q2[0:64, j, 0, :], in0=ps[0:64],
                        scalar1=bqh[0:64, j: j + 1], scalar2=None, op0=ALU.add,
                    )
                    nc.vector.tensor_scalar(
                        out=q2[64:128, j, 1, :], in0=ps[64:128],
                        scalar1=bqh[64:128, j: j + 1], scalar2=None, op0=ALU.add,
                    )
                else:
                    nc.vector.tensor_copy(out=q2[0:64, j, 0, :], in_=ps[0:64])
                    nc.vector.tensor_copy(out=q2[64:128, j, 1, :], in_=ps[64:128])
            # kT[d, w] = Wk_eff @ xnT  (all 576 halo tokens; k-bias dropped)
            for j in range(4):
                ps = pj.tile([128, 512], f32, tag="pj")
                ps2 = pj.tile([128, 512], f32, tag="pj")
                for c in range(4):
                    nc.tensor.matmul(
                        ps[:], ws["wk"][:, c, 128 * j: 128 * (j + 1)],
                        xnT[:, c, 0:512],
                        start=(c == 0), stop=(c == 3),
                    )
                for c in range(4):
                    nc.tensor.matmul(
                        ps2[:, 0:64], ws["wk"][:, c, 128 * j: 128 * (j + 1)],
                        xnT[:, c, 512:576],
                        start=(c == 0), stop=(c == 3),
                    )
                nc.scalar.activation(out=kT[:, j, 0:512], in_=ps[:], func=AF.Copy)
                nc.scalar.activation(
                    out=kT[:, j, 512:576], in_=ps2[:, 0:64], func=AF.Copy
                )
            # v token-major (feature-permuted Wv; bias folded into bo/bg)
            for t in range(5):
                rows = 128 if t < 4 else 64
                ps = pj.tile([128, 512], f32, tag="pj")
                for c in range(4):
                    nc.tensor.matmul(
                        ps[:rows], xnT[:, c, 128 * t: 128 * t + rows],
                        ws["wv"][:, c, :],
                        start=(c == 0), stop=(c == 3),
                    )
                nc.scalar.activation(
                    out=v_sb[:rows, t, :], in_=ps[:rows], func=AF.Copy
                )

            # ---- stage 4: software-pipelined banded attention ----
            # stage st emits scores+exp for block st, then the AV/normalize
            # consumers for block st-1, then block st's masks, then block
            # st-1's epilogue.  The in-order PE queue always has independent
            # work while the softmax chain of the other block drains, and the
            # DVE sees the normalize ops ahead of the next masks.
            a_tiles = {}
            for st in range(NBLK + 1):
                if st < NBLK:
                    b = st
                    for g in range(2):
                        s0 = sp0.tile([128, 2, 256], f32, tag="s0")
                        s1 = sp1.tile([64, 2, 256], f32, tag="s1")
                        for pr in range(2):
                            p = 2 * g + pr
                            q_ap = q2[:, p, :, 128 * b: 128 * (b + 1)]
                            nc.tensor.matmul(
                                s0[:, pr, :],
                                kT[:, p, 128 * b: 128 * b + 128],
                                q_ap, start=True, stop=True,
                            )
                            nc.tensor.matmul(
                                s1[:, pr, :],
                                kT[:, p, 128 * b + 128: 128 * b + 192],
                                q_ap, start=True, stop=True,
                            )
                        a0 = attn_pool.tile([128, 2, 256], bf16, tag="a0")
                        a1 = attn_pool.tile([64, 2, 256], bf16, tag="a1")
                        # s1 first: its single PSUM buffer is the next
                        # score-matmul's dependency
                        nc.scalar.activation(
                            out=a1[:].rearrange("p a b -> p (a b)"),
                            in_=s1[:].rearrange("p a b -> p (a b)"), func=AF.Exp,
                        )
                        nc.scalar.activation(
                            out=a0[:].rearrange("p a b -> p (a b)"),
                            in_=s0[:].rearrange("p a b -> p (a b)"), func=AF.Exp,
                        )
                        a_tiles[(b, g)] = (a0, a1)
                if st >= 1:
                    bp_ = st - 1
                    cps = cp.tile([128, 4, 128], f32, tag="cps")
                    for g in range(2):
                        a0, a1 = a_tiles[(bp_, g)]
                        a0f = a0[:].rearrange("p a b -> p (a b)")
                        a1f = a1[:].rearrange("p a b -> p (a b)")
                        # denominators broadcast to every partition by an
                        # all-ones stationary
                        bc = bcp.tile([128, 512], f32, tag="bc")
                        nc.tensor.matmul(
                            bc[:], ones_sb[:], a0f, start=True, stop=False,
                        )
                        nc.tensor.matmul(
                            bc[:], ones_sb[0:64, :], a1f, start=False, stop=True,
                        )
                        # AV: V stationary -> ctx feature-major (head h in
                        # feature tile h%4, partition half h//4 = g)
                        po = 64 * g
                        for hh in range(4):
                            nc.tensor.matmul(
                                cps[po: po + 64, hh, :],
                                v_sb[:, bp_, 128 * hh + po: 128 * hh + po + 64],
                                a0[:, hh >> 1, 128 * (hh & 1): 128 * (hh & 1) + 128],
                                start=True, stop=False,
                            )
                            nc.tensor.matmul(
                                cps[po: po + 64, hh, :],
                                v_sb[0:64, bp_ + 1, 128 * hh + po: 128 * hh + po + 64],
                                a1[:, hh >> 1, 128 * (hh & 1): 128 * (hh & 1) + 128],
                                start=False, stop=True,
                            )
                        rbc = rp_pool.tile([64, 512], f32, tag="rbc")
                        nc.vector.reciprocal_approx_fast(
                            out=rbc[:], in_=bc[po: po + 64, :]
                        )
                        nc.vector.tensor_mul(
                            out=ctxT[po: po + 64, :, bp_, :],
                            in0=cps[po: po + 64, :, :],
                            in1=rbc[:].rearrange("p (a q) -> p a q", q=128),
                        )
                        del a_tiles[(bp_, g)]
                if st < NBLK:
                    b = st
                    for g in range(2):
                        a0, a1 = a_tiles[(b, g)]
                        nc.gpsimd.tensor_mul(
                            out=a1[:].rearrange("p a b -> p (a b)"),
                            in0=a1[:].rearrange("p a b -> p (a b)"),
                            in1=mB_sb[:, b, :],
                        )
                        nc.vector.tensor_mul(
                            out=a0[:].rearrange("p a b -> p (a b)"),
                            in0=a0[:].rearrange("p a b -> p (a b)"),
                            in1=mA_sb[:, b, :],
                        )
                if st >= 1:
                    b = st - 1
                    # ---- epilogue for block b: gate (x and ctx parts share
                    # one PSUM accumulation), O-proj, sigmoid, blend, store.
                    # Only Exp/Sigmoid touch ACT in this phase, so both
                    # tables stay resident (no reloads).
                    gacc = pj.tile([128, 512], f32, tag="pj", name=f"gacc{b}")
                    for c in range(4):
                        nc.tensor.matmul(
                            gacc[:], xt_sb[:, c, 128 * b: 128 * (b + 1)],
                            ws["wg1"][:, c, :],
                            start=(c == 0), stop=False,
                        )
                    for c in range(4):
                        nc.tensor.matmul(
                            gacc[:], ctxT[:, c, b, :], ws["wg2"][:, c, :],
                            start=False, stop=(c == 3),
                        )
                    ops = pj.tile([128, 512], f32, tag="pj")
                    for c in range(4):
                        nc.tensor.matmul(
                            ops[:], ctxT[:, c, b, :], ws["wo"][:, c, :],
                            start=(c == 0), stop=(c == 3),
                        )
                    diff = fin_pool.tile([128, 512], f32, tag="diff")
                    gate = fin_pool.tile([128, 512], f32, tag="gate")
                    outs = fin_pool.tile([128, 512], bf16, tag="outs")
                    if bo_nz:
                        nc.vector.tensor_add(
                            out=ops[:], in0=ops[:], in1=bo_bc[:]
                        )
                    nc.vector.tensor_sub(out=diff[:], in0=ops[:], in1=x_sb[:, b, :])
                    if bg_nz:
                        nc.vector.tensor_add(
                            out=gacc[:], in0=gacc[:], in1=bg_bc[:]
                        )
                    nc.scalar.activation(
                        out=gate[:], in_=gacc[:], func=AF.Sigmoid
                    )
                    # out = x + gate * (o - x), column halves pipeline the
                    # multiply/add/store tail
                    for hf in range(2):
                        hs = slice(256 * hf, 256 * (hf + 1))
                        nc.vector.tensor_mul(
                            out=diff[:, hs], in0=diff[:, hs], in1=gate[:, hs]
                        )
                        nc.vector.tensor_add(
                            out=outs[:, hs], in0=diff[:, hs], in1=x_sb[:, b, hs]
                        )
                        nc.sync.dma_start(
                            out=out_t[:].rearrange("(c p) d -> p c d", p=128)[:, b, hs],
                            in_=outs[:, hs],
                        )
    nc.compile()
    return nc


def _host_prep(inputs):
    """Fold LN gain/bias + scale + Wp + bv into weights, build per-core maps."""
    x = np.asarray(inputs["token_embeds"], np.float32)
    g = np.asarray(inputs["ln_g"], np.float32)
    lb = np.asarray(inputs["ln_b"], np.float32)
    Wp = np.asarray(inputs["Wp"], np.float32)
    Wq = np.asarray(inputs["Wq"], np.float32)
    Wk = np.asarray(inputs["Wk"], np.float32)
    Wv = np.asarray(inputs["Wv"], np.float32)
    Wo = np.asarray(inputs["Wo"], np.float32)
    Wg = np.asarray(inputs["Wg"], np.float32)
    bp = np.asarray(inputs["bp"], np.float32)
    bq = np.asarray(inputs["bq"], np.float32)
    bv = np.asarray(inputs["bv"], np.float32)
    bo = np.asarray(inputs["bo"], np.float32)
    bg = np.asarray(inputs["bg"], np.float32)

    scale = 1.0 / np.sqrt(np.float32(DH))
    # feature permutation for ctx: head h features -> tile h%4, half h//4
    perm = np.zeros(D, np.int64)
    for h in range(H):
        c, gg = h % 4, h // 4
        perm[128 * c + 64 * gg: 128 * c + 64 * gg + 64] = np.arange(
            64 * h, 64 * h + 64
        )

    Wpq = (Wq @ Wp) * scale                       # folded q projection
    wq = np.ascontiguousarray((Wpq * g[None, :]).T).astype(BF16)
    wk = np.ascontiguousarray((Wk * g[None, :]).T).astype(BF16)
    wv_p = (Wv * g[None, :])[perm, :]             # permuted output features
    wv = np.ascontiguousarray(wv_p.T).astype(BF16)
    wo = np.ascontiguousarray(Wo[:, perm].T).astype(BF16)
    wg1 = np.ascontiguousarray(Wg[:, :D].T).astype(BF16)
    # reference gates on ctx AFTER the O-projection; fold Wo into Wg2 so the
    # gate matmul can consume pre-projection (permuted) ctx directly
    Wg2o = Wg[:, D:] @ Wo
    wg2 = np.ascontiguousarray(Wg2o[:, perm].T).astype(BF16)

    bq_eff = (Wq @ (Wp @ lb + bp) + bq) * scale
    bv_eff = Wv @ lb + bv
    # device ctx omits the v-bias; it re-enters as a constant through both
    # the O-projection and the folded gate projection
    bo_eff = Wo @ bv_eff + bo
    bg_eff = Wg[:, D:] @ bo_eff + bg

    bqh = np.ascontiguousarray(bq_eff.reshape(4, 128).T).astype(np.float32)
    flags = (
        bool(np.any(bq_eff != 0)),
        bool(np.any(bo_eff != 0)),
        bool(np.any(bg_eff != 0)),
    )

    in_maps = []
    for core in range(NCORES):
        bi, ci = core // 4, core % 4
        s = ci * CHUNK
        xr = np.zeros((XROWS, D), BF16)
        xr[0:CHUNK] = x[bi, s: s + CHUNK]
        if s - WCTX >= 0:
            xr[CHUNK: CHUNK + WCTX] = x[bi, s - WCTX: s]
        if s + CHUNK + WCTX <= T:
            xr[CHUNK + WCTX: CHUNK + 2 * WCTX] = x[bi, s + CHUNK: s + CHUNK + WCTX]
        xt = np.ascontiguousarray(x[bi, s: s + CHUNK].T).astype(BF16)

        # mask[b, rr, cc]: query r=128b+rr (local), key halo pos j=128b+cc;
        # duplicated 4x along columns (pair x evenodd) so the on-device
        # multiply is a contiguous 2D bf16 op
        rr = np.arange(128)[:, None]
        cc = np.arange(192)[None, :]
        m = np.zeros((NBLK, 128, 192), np.float32)
        for qb in range(NBLK):
            band = (cc - rr >= 0) & (cc - rr <= 2 * WCTX)
            gkey = s + 128 * qb + cc - WCTX + 0 * rr
            m[qb] = (band & (gkey >= 0) & (gkey < T)).astype(np.float32)
        mA = np.ascontiguousarray(
            np.tile(m[:, :, :128].transpose(2, 0, 1), (1, 1, 4))
        ).astype(BF16)
        mB = np.ascontiguousarray(
            np.tile(m[:, :, 128:].transpose(2, 0, 1), (1, 1, 4))
        ).astype(BF16)

        in_maps.append({
            "x": xr, "xt": xt, "mA": mA, "mB": mB,
            "wq": wq, "wk": wk, "wv": wv, "wo": wo,
            "wg1": wg1, "wg2": wg2,
            "bqh": bqh,
            "bo": bo_eff.reshape(1, D).astype(np.float32),
            "bg": bg_eff.reshape(1, D).astype(np.float32),
        })
    return in_maps, flags


def _run(inputs, trace=False):
    from concourse.bass_utils import run_bass_kernel_spmd

    in_maps, flags = _host_prep(inputs)
    if flags not in _CACHE:
        _CACHE[flags] = _build_program(flags)
    nc = _CACHE[flags]
    res = run_bass_kernel_spmd(nc, in_maps, list(range(NCORES)), trace=trace)
    out = np.zeros((B, T, D), np.float32)
    for core in range(NCORES):
        bi, ci = core // 4, core % 4
        out[bi, ci * CHUNK: (ci + 1) * CHUNK] = np.asarray(
            res.results[core]["out"], dtype=np.float32
        )
    return out, res


def kernel(**inputs):
    out, _ = _run(inputs, trace=False)
    return out
